# revision 1
# baseline (speedup 1.0000x reference)
"""Trainium2 Bass kernel for nn_ConditionalRandomField loss.

Strategy (data-parallel over batch, 64 sequences per core):

The CRF log-partition forward recursion is run in the EXP domain so each
timestep is one PE matmul + one DVE multiply:

    p_{t+1}[j,b] = (sum_i  exp(trans[i,j]-C) * p_t[i,b]) * exp(emit_{t+1}[j,b])

The gold-path (numerator) score is computed by the SAME recursion with the
emission tiles masked to the gold tag (one-hot), so the matmul weights
deliver the transition scores.  Both states are stacked on the free dim:
X = [p | g] as [64(j), 128] bf16, giving one matmul + one tensor-tensor
multiply per timestep.  The sequence is processed from both ends at once
(forward chain t=0..511, backward chain t=1023..512, merged in the middle)
so two independent dependency chains keep the engines busy.

Normalization: a global constant C is folded into the weights; every 32
steps a per-sequence rescale r = 1/S (S = column sum, obtained from an
extra row-sum column appended to the weights) is folded into an upcoming
emission tile; log r is accumulated exactly at the end via ACT log of the
columnized r values.

Mask is all ones per the problem spec (fill: ones); asserted host-side.
"""

import numpy as np

import concourse.bass as bass
import concourse.tile as tile
from concourse import bacc, mybir
from concourse.bass_utils import run_bass_kernel_spmd

F32 = mybir.dt.float32
BF16 = mybir.dt.bfloat16
I32 = mybir.dt.int32
Alu = mybir.AluOpType
Act = mybir.ActivationFunctionType

B, T, K = 512, 1024, 64
NCORES = 8
BL = B // NCORES            # 64 sequences per core
CNORM = 4.67                # global per-step growth constant, folded into weights
BIG = 50.0                  # one-hot mask log-offset
NORM = 32                   # rescale period (steps)
LAG = 4                     # rescale uses S from LAG steps earlier
M = T // 2 - 1              # forward chain covers t=0..M (M matmul steps)
NB = T - 1 - M              # backward chain matmul steps
W = 32                      # timesteps per prep window
NWIN = T // W
TPK = 8                     # timesteps per pack tile
N_EV_F = M // NORM          # 15 forward rescale events
N_EV_B = NB // NORM         # 16 backward rescale events
N_EV = N_EV_F + N_EV_B


def rep_dim(ap, pos, n):
    """Insert a step-0 (broadcast) dim of size n at position pos of an AP."""
    dims = [list(d) for d in ap.ap]
    dims.insert(pos, [0, n])
    return bass.AP(tensor=ap.tensor, offset=ap.offset, ap=dims)


def build_nc():
    nc = bacc.Bacc("TRN2", target_bir_lowering=False, debug=False)

    x = nc.dram_tensor("x", [BL, T, K], F32, kind="ExternalInput")
    tg = nc.dram_tensor("tg", [BL, T], I32, kind="ExternalInput")
    trans = nc.dram_tensor("trans", [K, K], F32, kind="ExternalInput")
    startc = nc.dram_tensor("startc", [K, 1], F32, kind="ExternalInput")
    endc = nc.dram_tensor("endc", [K, 1], F32, kind="ExternalInput")
    out = nc.dram_tensor("out", [1, 1], F32, kind="ExternalOutput")

    from contextlib import ExitStack
    with tile.TileContext(nc) as tc, ExitStack() as ctx:
        con = ctx.enter_context(tc.tile_pool(name="con", bufs=1))
        xwp = ctx.enter_context(tc.tile_pool(name="xwp", bufs=2))
        ohp = ctx.enter_context(tc.tile_pool(name="ohp", bufs=2))
        gmp = ctx.enter_context(tc.tile_pool(name="gmp", bufs=2))
        pkp = ctx.enter_context(tc.tile_pool(name="pkp", bufs=24))
        stp = ctx.enter_context(tc.tile_pool(name="stp", bufs=6))
        evp = ctx.enter_context(tc.tile_pool(name="evp", bufs=4))
        fin = ctx.enter_context(tc.tile_pool(name="fin", bufs=1))
        ps_qf = ctx.enter_context(tc.tile_pool(name="ps_qf", bufs=2, space="PSUM"))
        ps_qb = ctx.enter_context(tc.tile_pool(name="ps_qb", bufs=2, space="PSUM"))
        ps_tr = ctx.enter_context(tc.tile_pool(name="ps_tr", bufs=2, space="PSUM"))
        ps_rl = ctx.enter_context(tc.tile_pool(name="ps_rl", bufs=1, space="PSUM"))
        ps_ms = ctx.enter_context(tc.tile_pool(name="ps_ms", bufs=1, space="PSUM"))

        # ---------------- constants ----------------
        ident = con.tile([K, K], F32)
        onetile = con.tile([K, K], F32)
        nc.vector.memset(onetile[:], 1.0)
        nc.vector.memset(ident[:], 0.0)
        nc.gpsimd.affine_select(ident[:], onetile[:], pattern=[[-1, K]],
                                compare_op=Alu.is_equal, fill=0.0,
                                base=0, channel_multiplier=1)

        iota_i = con.tile([K, K], I32)
        nc.gpsimd.iota(iota_i[:], pattern=[[1, K]], base=0, channel_multiplier=0)
        iota_b = con.tile([K, K], BF16)
        nc.vector.tensor_copy(iota_b[:], iota_i[:])

        # tags -> bf16
        tg_i = con.tile([BL, T], I32)
        nc.sync.dma_start(tg_i[:], tg[:])
        tg_b = con.tile([BL, T], BF16)
        nc.vector.tensor_copy(tg_b[:], tg_i[:])

        # bias constant tiles (activation float biases need const APs)
        bias_mc64 = con.tile([K, 1], F32)
        nc.vector.memset(bias_mc64[:], -CNORM)
        bias_cb128 = con.tile([2 * K, 1], F32)
        nc.vector.memset(bias_cb128[:], CNORM - BIG)

        # E = exp(trans - C); Ebf = [E | rowsum] bf16; EbT likewise for E^T
        tr_t = con.tile([K, K], F32)
        nc.sync.dma_start(tr_t[:], trans[:])
        e_f = con.tile([K, K], F32)
        nc.scalar.activation(e_f[:], tr_t[:], Act.Exp, bias=bias_mc64[:])
        e_bf = con.tile([K, K + 1], BF16)
        nc.vector.tensor_copy(e_bf[:, 0:K], e_f[:])
        rs_f = con.tile([K, 1], F32)
        nc.vector.tensor_reduce(rs_f[:], e_f[:], axis=mybir.AxisListType.X,
                                op=Alu.add)
        nc.vector.tensor_copy(e_bf[:, K:K + 1], rs_f[:])

        p_et = ps_ms.tile([K, K], F32, tag="misc")
        nc.tensor.transpose(p_et[:], e_f[:], ident[:])
        et_f = con.tile([K, K], F32)
        nc.vector.tensor_copy(et_f[:], p_et[:])
        et_bf = con.tile([K, K + 1], BF16)
        nc.vector.tensor_copy(et_bf[:, 0:K], et_f[:])
        rs_b = con.tile([K, 1], F32)
        nc.vector.tensor_reduce(rs_b[:], et_f[:], axis=mybir.AxisListType.X,
                                op=Alu.add)
        nc.vector.tensor_copy(et_bf[:, K:K + 1], rs_b[:])

        start_t = con.tile([K, 1], F32)
        nc.sync.dma_start(start_t[:], startc[:])
        end_t = con.tile([K, 1], F32)
        nc.sync.dma_start(end_t[:], endc[:])
        exp_start = con.tile([K, 1], F32)
        nc.scalar.activation(exp_start[:], start_t[:], Act.Exp)
        exp_start_mc = con.tile([K, 1], F32)
        nc.scalar.activation(exp_start_mc[:], start_t[:], Act.Exp, bias=bias_mc64[:])

        ones_row = con.tile([1, K], F32)
        nc.vector.memset(ones_row[:], 1.0)
        ones_1 = con.tile([1, 1], F32)
        nc.vector.memset(ones_1[:], 1.0)
        ones_col_b = con.tile([K, 1], BF16)
        nc.vector.memset(ones_col_b[:], 1.0)
        zeros_kk = con.tile([K, 2 * BL], F32)
        nc.vector.memset(zeros_kk[:], 0.0)
        sign_col = con.tile([2 * BL, 1], F32)
        nc.vector.memset(sign_col[:], 1.0)
        nc.vector.memset(sign_col[0:BL, :], -1.0)

        # ---------------- window prep ----------------
        # pack tile layout: [64(j), TPK(t), 2(m|mh), 64(b)] bf16, partitions 0-63
        packs = {}   # t//TPK -> tile

        def prep_window(w):
            t0 = w * W
            xw = xwp.tile([BL, W * K], F32, tag="xw")
            nc.sync.dma_start(xw[:], x[:, t0:t0 + W, :].rearrange("b t k -> b (t k)"))
            oh = ohp.tile([BL, W, K], BF16, tag="oh")
            nc.vector.tensor_tensor(out=oh[:], in0=rep_dim(iota_b[:], 1, W),
                                    in1=rep_dim(tg_b[:, t0:t0 + W], 2, K),
                                    op=Alu.is_equal)
            gm = gmp.tile([BL, W * K], F32, tag="gm")
            nc.vector.scalar_tensor_tensor(
                out=gm[:], in0=xw[:], scalar=BIG,
                in1=oh[:].rearrange("b t k -> b (t k)"),
                op0=Alu.add, op1=Alu.mult)
            for pk in range(W // TPK):         # packs per window
                pack = pkp.tile([K, TPK, 2, BL], BF16, tag="pack")
                praw = ps_tr.tile([K, TPK, BL], F32, tag="ptr")
                pgm = ps_tr.tile([K, TPK, BL], F32, tag="ptr")
                for tl in range(TPK):          # one transpose per timestep
                    off = (pk * TPK + tl) * K
                    nc.tensor.transpose(praw[:, tl, :], xw[:, off:off + K],
                                        ident[:])
                    nc.tensor.transpose(pgm[:, tl, :], gm[:, off:off + K],
                                        ident[:])
                nc.scalar.activation(pack[:, :, 0, :], praw[:], Act.Exp)
                nc.scalar.activation(pack[:, :, 1, :], pgm[:], Act.Exp,
                                     bias=bias_cb128[0:K, :])
                packs[t0 // TPK + pk] = pack

        def mtile(t):
            """[64(j), 128(b_m | b_mh)] bf16 slice for timestep t."""
            pack = packs[t // TPK]
            u = t % TPK
            return pack[:, u, :, :].rearrange("p a b -> p (a b)")

        # PSUM tile [128, 31]: columnized rescale factors (fwd 0:15, bwd 15:31)
        p_rlog = ps_rl.tile([2 * BL, N_EV], F32)

        prep_window(0)
        prep_window(NWIN - 1)

        # ---------------- chain inits ----------------
        xf = stp.tile([K, 2 * BL], BF16, tag="xf")
        m0 = mtile(0)
        nc.vector.tensor_scalar_mul(xf[:, 0:BL], m0[:, 0:BL], exp_start[:])
        nc.vector.tensor_scalar_mul(xf[:, BL:2 * BL], m0[:, BL:2 * BL],
                                    exp_start_mc[:])
        xb = stp.tile([K, 2 * BL], BF16, tag="xb")
        nc.scalar.activation(xb[:], zeros_kk[:], Act.Exp, bias=end_t[:])

        def event_prep(q, ev_idx, target_m):
            """Fold r = 1/S (row 64 of q) into target_m; columnize r for logC."""
            s_sb = evp.tile([1, 2 * BL], F32, tag="srow")
            nc.scalar.copy(s_sb[:], q[K:K + 1, :])
            r_row = evp.tile([1, 2 * BL], F32, tag="rrow")
            nc.vector.reciprocal(r_row[:], s_sb[:])
            nc.tensor.matmul(p_rlog[:, ev_idx:ev_idx + 1], r_row[:], ones_1[:],
                             start=True, stop=True)
            p_bc = ps_ms.tile([K, 2 * BL], F32, tag="misc")
            nc.tensor.matmul(p_bc[:], ones_row[:], r_row[:], start=True, stop=True)
            nc.vector.tensor_tensor(out=target_m, in0=target_m, in1=p_bc[:],
                                    op=Alu.mult)

        # ---------------- the two chains ----------------
        qb_prev = None
        for s in range(1, NB + 1):
            # window prefetch: 24 steps before a chain needs its next window
            if s % W == 8 and s + 24 <= M:
                prep_window((s + 24) // W)
            if s % W == 8 and s + 24 < NB:
                prep_window(NWIN - 1 - (s + 24) // W)

            # forward step s (consumes emission t=s)
            if s <= M:
                qf = ps_qf.tile([K + 1, 2 * BL], F32, tag="qf")
                nc.tensor.matmul(qf[:], e_bf[:], xf[:], start=True, stop=True)
                if (s + LAG) % NORM == 0 and s + LAG <= M:
                    event_prep(qf, (s + LAG) // NORM - 1, mtile(s + LAG))
                xf = stp.tile([K, 2 * BL], BF16, tag="xf")
                nc.vector.tensor_tensor(out=xf[:], in0=qf[0:K, :], in1=mtile(s),
                                        op=Alu.mult)

            # backward step s (consumes emission t=T-s)
            ub = stp.tile([K, 2 * BL], BF16, tag="ub")
            bin0 = xb[:] if s == 1 else qb_prev[0:K, :]
            nc.vector.tensor_tensor(out=ub[:], in0=bin0, in1=mtile(T - s),
                                    op=Alu.mult)
            qb = ps_qb.tile([K + 1, 2 * BL], F32, tag="qb")
            nc.tensor.matmul(qb[:], et_bf[:], ub[:], start=True, stop=True)
            qb_prev = qb
            if (s + LAG) % NORM == 0 and s + LAG <= NB:
                event_prep(qb, N_EV_F + (s + LAG) // NORM - 1,
                           mtile(T - (s + LAG)))

        # ---------------- merge + finals ----------------
        w_z = fin.tile([K, 2 * BL], BF16)
        nc.vector.tensor_tensor(out=w_z[:], in0=qb_prev[0:K, :], in1=xf[:],
                                op=Alu.mult)
        p_ms = ps_ms.tile([2 * BL, 1], F32, tag="misc")
        nc.tensor.matmul(p_ms[:], w_z[:], ones_col_b[:], start=True, stop=True)
        l_t = fin.tile([2 * BL, 1], F32)
        nc.scalar.activation(l_t[:], p_ms[:], Act.Ln)
        rlog_sb = fin.tile([2 * BL, N_EV], F32)
        nc.vector.tensor_copy(rlog_sb[:], p_rlog[:])
        rlog_l = fin.tile([2 * BL, N_EV], F32)
        nc.scalar.activation(rlog_l[:], rlog_sb[:], Act.Ln)
        rl_sum = fin.tile([2 * BL, 1], F32)
        nc.vector.tensor_reduce(rl_sum[:], rlog_l[:], axis=mybir.AxisListType.X,
                                op=Alu.add)
        v_t = fin.tile([2 * BL, 1], F32)
        nc.vector.tensor_tensor(out=v_t[:], in0=l_t[:], in1=rl_sum[:],
                                op=Alu.subtract)
        p_tot = ps_ms.tile([1, 1], F32, tag="misc")
        nc.tensor.matmul(p_tot[:], v_t[:], sign_col[:], start=True, stop=True)
        out_sb = fin.tile([1, 1], F32)
        nc.vector.tensor_scalar_add(out_sb[:], p_tot[:],
                                    -float(BL * (T - 1)) * CNORM)
        nc.sync.dma_start(out[:], out_sb[:])

    nc.compile()
    return nc


_NC_CACHE = None


def kernel(**inputs) -> np.ndarray:
    global _NC_CACHE
    logits = np.ascontiguousarray(np.asarray(inputs["inputs"], dtype=np.float32))
    tags = np.asarray(inputs["tags"]).astype(np.int32)
    mask = np.asarray(inputs["mask"])
    trans = np.ascontiguousarray(np.asarray(inputs["transitions"], np.float32))
    start = np.asarray(inputs["start_transitions"], np.float32).reshape(K, 1)
    end = np.asarray(inputs["end_transitions"], np.float32).reshape(K, 1)
    assert mask.min() == 1, "kernel assumes mask of all ones (spec fill=ones)"

    if _NC_CACHE is None:
        _NC_CACHE = build_nc()
    nc = _NC_CACHE

    trans_c = np.ascontiguousarray(trans)
    in_maps = []
    for c in range(NCORES):
        sl = slice(c * BL, (c + 1) * BL)
        in_maps.append({
            "x": np.ascontiguousarray(logits[sl]),
            "tg": np.ascontiguousarray(tags[sl]),
            "trans": trans_c,
            "startc": start,
            "endc": end,
        })
    res = run_bass_kernel_spmd(nc, in_maps, core_ids=list(range(NCORES)))
    total = np.float64(0.0)
    for c in range(NCORES):
        total += np.float64(res.results[c]["out"][0, 0])
    return np.float32(total)

